# revision 49
# baseline (speedup 1.0000x reference)
"""LoraLinear (x @ W.T + 2*(x @ A.T) @ B.T) on 8 TRN2 NeuronCores.

Tensor-parallel: W and lora_B sharded row-wise (out_features) across 8
cores; x and lora_A replicated. The dominant HBM stream (W shard) is
quantized host-side to fp8 e4m3 (scaled x64 so ~N(0,1) values sit in
e4m3's normal range): 32 -> 8.4 MiB per core.

x is quantized to e4m3 as the DoubleRow stationary ([128k, 2, 64t]
K-pair planes, two 128-K planes per matmul pass); the lora path stays
bf16 (it dominates output variance; fp8 there would blow the error
budget). Everything lands 64x scaled in psum, goes out as f16 (values
~N(0, 200), far inside f16 range) and the host divides once at the
end. Measured quantization error 1.22e-2 Frobenius vs the 2e-2 gate,
deterministic for the fixed-seed inputs.

Streaming layout tuned from neuron-profile traces: DMA runs at rate
(~420 GB/s) only for homogeneous fat-line 128-partition transfers --
64-row or thin-line transfers crawl. So: xs/xt/at/bt are packed into
ONE byte-blob transfer (SBUF views recover dtypes via bitcast; bt is
column-split across partition halves), followed by 16 identical 512
KiB W slabs, and the output leaves in ONE 128-partition transfer (the
psum->SBUF copies relocate banks 2,3 into out_sb partitions 64:127;
the host reassembles the column blocks). All slabs stay resident in
SBUF so the stream never stalls on compute. The lora-u matmuls run
early (slab 2) inside the DMA shadow; the output-bank copies are split
between DVE and the Scalar engine (whose activation table is prewarmed
by a dummy copy so the one-time ACT_TABLE_LOAD stays out of the tail).

DMA completion semaphores increment +1 per DMA engine (16 per
transfer) and counts from different transfers mix, so every transfer
that gates compute gets its own semaphore and waits use full totals.

Self-contained: shapes hardcoded for
  x [64, 4096] f32, weight [16384, 4096] f32,
  lora_A [64, 4096] f32, lora_B [16384, 64] f32  ->  out [64, 16384] f32
"""

import ml_dtypes
import numpy as np

import concourse.bass as bass
import concourse.mybir as mybir
from concourse.bass_utils import run_bass_kernel_spmd

N_CORES = 8
TOK = 64          # tokens
IN_F = 4096       # in_features (contraction)
OUT_F = 16384     # out_features
R = 64            # lora rank
SCALING = 2.0
WSCALE = 64.0     # fp8 pre-scale for W (folded out on host)
O_SHARD = OUT_F // N_CORES   # 2048 out features per core
P = 128
KT = IN_F // P               # 32 k-subtiles of 128
KP = KT // 2                 # 16 DoubleRow pair-slabs of 256 K
NB = O_SHARD // 512          # 4 psum blocks of 512
IB = 12288                   # input blob bytes per partition (xs|xt|at|bt)
F32 = mybir.dt.float32
F16 = mybir.dt.float16
BF16 = mybir.dt.bfloat16
FP8 = mybir.dt.float8e4
NPBF = ml_dtypes.bfloat16
NPF8 = ml_dtypes.float8_e4m3

UT_AFTER_SLAB = 2            # run the lora-u matmuls after this slab
LORA_AFTER_SLAB = 4          # add the lora epilogue after this slab


def _build_nc():
    nc = bass.Bass()
    # Host-prepared layouts (see _prep_in_maps):
    #   inp [128, 12288] byte blob: cols 0:2048 xs (fp8 e4m3(x.T) k-tiles),
    #       2048:6144 xt (bf16 x.T k-tiles), 6144:10240 at (bf16
    #       (2*WSCALE*A).T k-tiles), 10240:12288 bt (bf16 lora_B.T shard,
    #       column-split: rows 0:63 = bt[:, 0:1024], rows 64:127 =
    #       bt[:, 1024:2048] -- no 64-row transfer, no padding)
    #   wt  [2048, 4096]  fp8 e4m3(WSCALE*W.T) shard; slab j row p =
    #                     concat(w[256j+p, :], w[256j+128+p, :]) (pair planes)
    inp = nc.dram_tensor("inp", [P, IB], FP8, kind="ExternalInput")
    wt = nc.dram_tensor("wt", [15 * P, 2 * O_SHARD], FP8, kind="ExternalInput")
    # slab 15 column-split into two half transfers (2 KiB lines measure
    # ~95% of 4 KiB rate): banks 0,1 finish one transfer early so their
    # matmuls+copies hide under the final transfer. Row p layout:
    # [h0: plane0 cols 0:1024 | plane1 cols 0:1024 | h1: ...cols 1024:2048]
    wt15 = nc.dram_tensor("wt15", [P, 2 * O_SHARD], FP8, kind="ExternalInput")
    # out is 64x scaled; host divides. f16: values ~N(0, 200), far in range.
    # Column-split like bt: rows 0:63 = out cols (0:512, 512:1024) and rows
    # 64:127 = cols (1024:1536, 1536:2048) -- a 128-partition transfer
    # (64-row transfers crawl); host reassembles.
    out = nc.dram_tensor("out", [P, O_SHARD // 2], F16, kind="ExternalOutput")

    with (
        nc.sbuf_tensor("in_sb", [P, IB], FP8) as in_sb,
        nc.sbuf_tensor("ut_sb", [P, TOK], BF16) as ut_sb,
        nc.sbuf_tensor("w_sb", [P, KP - 1, 2, O_SHARD], FP8) as w_sb,
        nc.sbuf_tensor("w15_sb", [P, 2, 2, 1024], FP8) as w15_sb,
        nc.sbuf_tensor("out_sb", [P, O_SHARD // 2], F16) as out_sb,
        nc.sbuf_tensor("warm_sb", [1, 2], F32) as warm_sb,
        nc.psum_tensor("ps_o", [TOK, NB, 512], F32) as ps_o,
        nc.psum_tensor("ps_ut", [R, TOK], F32) as ps_ut,
        nc.semaphore("in_sem") as in_sem,     # input blob DMA done (+16)
        nc.semaphore("pe_sem") as pe_sem,     # PE milestones (+1)
        nc.semaphore("cp_sem") as cp_sem,     # DVE copies done (+1)
        nc.semaphore("act_sem") as act_sem,   # Scalar copies done (+1)
        nc.semaphore("done_sem") as done_sem, # out DMA done (+16 each)
        nc.Block() as block,
    ):
        w_sems = [nc.alloc_semaphore(name=f"w_sem{j}") for j in range(KP - 1)]
        w15_sems = [nc.alloc_semaphore(name=f"w15_sem{h}") for h in range(2)]

        # dtype views into the input blob
        xs_v = in_sb[:, 0:2048].rearrange("p (kt t) -> p kt t", kt=KT)
        xt_v = in_sb[:, 2048:6144].bitcast(BF16).rearrange(
            "p (kt t) -> p kt t", kt=KT)
        at_v = in_sb[:, 6144:10240].bitcast(BF16).rearrange(
            "p (kt t) -> p kt t", kt=KT)
        bt_lo = in_sb[0:R, 10240:12288].bitcast(BF16)    # bt cols 0:1024
        bt_hi = in_sb[R:P, 10240:12288].bitcast(BF16)    # bt cols 1024:2048

        def wslab(eng, j):
            eng.dma_start(
                out=w_sb[:, j, :, :],
                in_=wt[j * P:(j + 1) * P, :].rearrange(
                    "p (two o) -> p two o", two=2),
            ).then_inc(w_sems[j], 16)

        @block.sync
        def _(sync):
            sync.dma_start(out=in_sb[:], in_=inp[:]).then_inc(in_sem, 16)
            for j in range(KP - 1):
                wslab(sync, j)
            for h in range(2):
                sync.dma_start(
                    out=w15_sb[:, h, :, :],
                    in_=wt15[:, h * 2048:(h + 1) * 2048].rearrange(
                        "p (two o) -> p two o", two=2),
                ).then_inc(w15_sems[h], 16)
            # single 128-partition out DMA (64-row transfers crawl)
            sync.wait_ge(cp_sem, 4)            # 2 ut copies + DVE banks 0,2
            sync.wait_ge(act_sem, 2)           # Scalar banks 1,3
            sync.dma_start(out=out[:], in_=out_sb[:]).then_inc(done_sem, 16)
            sync.wait_ge(done_sem, 16)

        @block.tensor
        def _(tensor):
            tensor.wait_ge(in_sem, 16)         # xs/xt/at resident
            for j in range(KP - 1):
                tensor.wait_ge(w_sems[j], 16)
                for b in range(NB):
                    nc.tensor.matmul(
                        ps_o[:, b, :],
                        xs_v[:, 2 * j:2 * j + 2, :],
                        w_sb[:, j, :, b * 512:(b + 1) * 512],
                        start=(j == 0), stop=False,
                        perf_mode=mybir.MatmulPerfMode.DoubleRow,
                        skip_group_check=True)
                if j == UT_AFTER_SLAB:
                    # lora uT = (2*WSCALE*A) @ x.T in bf16, early so the
                    # contiguous ~6 us block also ramps the PE p-state:
                    # lhsT = at tile [128k, 64r], rhs = xt tile [128k, 64t]
                    for k in range(KT):
                        mmu = nc.tensor.matmul(
                            ps_ut[:], at_v[:, k, :], xt_v[:, k, :],
                            start=(k == 0), stop=(k == KT - 1))
                    mmu.then_inc(pe_sem, 1)
                if j == LORA_AFTER_SLAB:
                    # lora: psum += uT.T @ bT (all 64x scaled), mid-stream
                    # so the tail is just slab 15 + copies. bt is column-
                    # split across partition halves in the blob; ut is
                    # replicated into partitions 64:127 so banks 2,3 read
                    # both operands at base partition 64.
                    tensor.wait_ge(cp_sem, 2)  # both ut copies done
                    for b in range(NB):
                        lhs = ut_sb[0:R, :] if b < 2 else ut_sb[R:P, :]
                        rhs = (bt_lo if b < 2 else bt_hi)[
                            :, (b % 2) * 512:(b % 2 + 1) * 512]
                        nc.tensor.matmul(
                            ps_o[:, b, :], lhs, rhs,
                            start=False, stop=False,
                            skip_group_check=True)

            # slab 15: banks 0,1 right after the first half transfer
            for h in range(2):
                tensor.wait_ge(w15_sems[h], 16)
                for c in range(2):
                    b = 2 * h + c
                    nc.tensor.matmul(
                        ps_o[:, b, :],
                        xs_v[:, 30:32, :],
                        w15_sb[:, h, :, c * 512:(c + 1) * 512],
                        start=False, stop=True,
                        perf_mode=mybir.MatmulPerfMode.DoubleRow,
                        skip_group_check=True,
                    ).then_inc(pe_sem, 1)

        @block.vector
        def _(vector):
            vector.wait_ge(pe_sem, 1)          # ut accumulation done
            nc.vector.tensor_copy(out=ut_sb[0:R, :], in_=ps_ut[:]).then_inc(
                cp_sem, 1)
            nc.vector.tensor_copy(out=ut_sb[R:P, :], in_=ps_ut[:]).then_inc(
                cp_sem, 1)
            # pe_sem 2..5: slab 15's bank-b matmul retired. Copies relocate
            # banks 2,3 into out_sb partitions 64:127 so the out transfer is
            # 128 partitions wide (64-row transfers crawl).
            vector.wait_ge(pe_sem, 2)
            nc.vector.tensor_copy(
                out=out_sb[0:TOK, 0:512], in_=ps_o[:, 0, :]).then_inc(cp_sem, 1)
            vector.wait_ge(pe_sem, 4)
            nc.vector.tensor_copy(
                out=out_sb[TOK:P, 0:512], in_=ps_o[:, 2, :]).then_inc(cp_sem, 1)

        @block.scalar
        def _(scalar):
            # dummy copy at thread start: forces the one-time ACT_TABLE_LOAD
            # (~1.3 us) to happen during the DMA stream, not in the tail
            nc.scalar.copy(out=warm_sb[0:1, 1:2], in_=warm_sb[0:1, 0:1])
            scalar.wait_ge(pe_sem, 3)
            nc.scalar.copy(
                out=out_sb[0:TOK, 512:1024], in_=ps_o[:, 1, :]).then_inc(
                act_sem, 1)
            scalar.wait_ge(pe_sem, 5)
            nc.scalar.copy(
                out=out_sb[TOK:P, 512:1024], in_=ps_o[:, 3, :]).then_inc(
                act_sem, 1)

    return nc


_NC_CACHE = None


def _get_nc():
    global _NC_CACHE
    if _NC_CACHE is None:
        _NC_CACHE = _build_nc()
    return _NC_CACHE


def _ktile(a):
    # [4096, T] -> partition-major k-tile layout [128, KT*T]
    t = a.shape[1]
    return np.ascontiguousarray(
        a.reshape(KT, P, t).transpose(1, 0, 2).reshape(P, KT * t))


def _prep_in_maps(x, weight, lora_A, lora_B):
    xT = np.ascontiguousarray(x.T)                       # [4096, 64]
    xs = _ktile(xT).astype(NPF8)                         # [128, 2048] fp8
    xt = _ktile(xT).astype(NPBF)
    at = _ktile(np.ascontiguousarray((SCALING * WSCALE * lora_A).T)).astype(NPBF)
    base = np.empty((P, IB), dtype=np.uint8)
    base[:, 0:2048] = xs.view(np.uint8)
    base[:, 2048:6144] = np.ascontiguousarray(xt).view(np.uint8)
    base[:, 6144:10240] = np.ascontiguousarray(at).view(np.uint8)
    wq_full = (WSCALE * weight.T).astype(NPF8)           # [4096, 16384] fp8
    bt_full = np.ascontiguousarray(lora_B.T).astype(NPBF)  # [64, 16384]
    in_maps = []
    for c in range(N_CORES):
        sl = slice(c * O_SHARD, (c + 1) * O_SHARD)
        blob = base.copy()
        # bt column-split across partition halves (see _build_nc comment)
        btb = np.ascontiguousarray(bt_full[:, sl]).view(np.uint8)  # [64,4096]
        blob[0:R, 10240:12288] = btb[:, 0:2048]
        blob[R:P, 10240:12288] = btb[:, 2048:4096]
        wc = wq_full[:, sl]                              # [4096, 2048]
        # pair planes: slab j row p = [w[256j+p], w[256j+128+p]]
        S = np.ascontiguousarray(
            wc.reshape(KP, 2, P, O_SHARD).transpose(0, 2, 1, 3)
            .reshape(KP, P, 2 * O_SHARD))
        wtm = np.ascontiguousarray(S[0:15].reshape(15 * P, 2 * O_SHARD))
        # slab 15 repack: [p, half, plane, 1024] so each half transfer has
        # 2 KiB contiguous lines on both DRAM and SBUF sides
        wt15 = np.ascontiguousarray(
            S[15].reshape(P, 2, 2, 1024).transpose(0, 2, 1, 3)
            .reshape(P, 2 * O_SHARD))
        in_maps.append({"inp": blob.view(NPF8), "wt": wtm, "wt15": wt15})
    return in_maps


def kernel(x, weight, lora_A, lora_B, trace=False):
    x = np.asarray(x, dtype=np.float32)
    weight = np.asarray(weight, dtype=np.float32)
    lora_A = np.asarray(lora_A, dtype=np.float32)
    lora_B = np.asarray(lora_B, dtype=np.float32)
    nc = _get_nc()
    in_maps = _prep_in_maps(x, weight, lora_A, lora_B)
    res = run_bass_kernel_spmd(nc, in_maps, core_ids=list(range(N_CORES)),
                               trace=trace)
    inv = np.float32(1.0 / WSCALE)
    outs = []
    for c in range(N_CORES):
        r = res.results[c]["out"].astype(np.float32) * inv   # [128, 1024]
        full = np.empty((TOK, O_SHARD), dtype=np.float32)
        for h in range(2):
            for cc in range(2):
                full[:, 512 * (2 * h + cc):512 * (2 * h + cc + 1)] = \
                    r[h * TOK:(h + 1) * TOK, cc * 512:(cc + 1) * 512]
        outs.append(full)
    out = np.concatenate(outs, axis=1)
    if trace:
        kernel.last_results = res
    return out
